# revision 12
# baseline (speedup 1.0000x reference)
"""HardTripletLoss Trainium2 kernel.

Reference computation (B=256, C=1000, D=300):
  relations[b,c] = ||emb[b*C+c] - att[b*C+c] + 1e-6||_2          [B, C]
  hardest_positive[c] = max_b relations[b,c] * onehot(labels)[b,c]
  mx[c]              = max_b relations[b,c]
  hardest_negative[c] = min_b (relations[b,c] + mx[c]*onehot[b,c])
  loss = sum(relu(hp - hn + 1)) / (count(relu(...) > 1e-16) + 1e-16)

Sharding: data-parallel over B across 8 cores (32 b's per core, each a
contiguous 32000-row chunk of the (B*C, D) tensors). Each core computes
squared distances and 4 per-class partial reductions [C]:
  cmax = max_b rel_sq            pmax = max_b over positives of rel_sq
  umin = min_b over negatives    mmin = min_b over positives
(masking is exact: +/-1e30 select-style masks via min/max ALU ops, no
additive-cancellation error). Host all-reduces the [4, C] partials over
cores, takes sqrt (monotone, commutes with max/min), and finishes the
tiny [C]-sized tail: hn = min(umin, cmax_r + mmin), loss scalar.

On-chip layout per core: c is tiled as c = j*125 + p (j in [0,8),
p = partition in [0,125)) so every per-(b,j) tile is [125 partitions,
300 free] and per-class reductions are free-axis reductions.
Per b: one DMA per tensor [125, (8,300)] (1.2 MB), one DVE subtract,
8 ACT Square(bias=1e-6) ops with accum_out -> rel_sq column.
"""

import numpy as np

B, C, D = 256, 1000, 300
M = 8            # cores
BL = B // M      # 32 local anchors per core
P = 125          # partition tile: c = j*P + p
J = C // P       # 8 c-blocks
BIG = 1.0e30
EPS_PD = 1e-6
MARGIN = 1.0

_STATE = {}


def _build():
    import concourse.tile as tile
    from concourse import bacc, mybir

    nc = bacc.Bacc("TRN2", target_bir_lowering=False, debug=False,
                   num_devices=M)
    dt = mybir.dt.float32
    emb = nc.dram_tensor("emb", [BL * C, D], dt, kind="ExternalInput").ap()
    att = nc.dram_tensor("att", [BL * C, D], dt, kind="ExternalInput").ap()
    msk = nc.dram_tensor("msk", [P, J * BL], dt, kind="ExternalInput").ap()
    out = nc.dram_tensor("out", [P, 4 * J], dt, kind="ExternalOutput").ap()

    emb_v = emb.rearrange("(b j p) d -> b p j d", b=BL, j=J, p=P)
    att_v = att.rearrange("(b j p) d -> b p j d", b=BL, j=J, p=P)

    Alu = mybir.AluOpType
    Act = mybir.ActivationFunctionType
    Ax = mybir.AxisListType

    with tile.TileContext(nc) as tc:
        with (
            tc.tile_pool(name="io", bufs=4) as io_pool,
            tc.tile_pool(name="dif", bufs=2) as dif_pool,
            tc.tile_pool(name="small", bufs=1) as small_pool,
        ):
            mask_t = small_pool.tile([P, J * BL], dt, tag="mask")
            nc.sync.dma_start(mask_t[:], msk[:])
            mask2_t = small_pool.tile([P, J * BL], dt, tag="mask2")
            nc.vector.tensor_scalar_mul(mask2_t[:], mask_t[:], -1.0)
            rel_t = small_pool.tile([P, J * BL], dt, tag="rel")
            junk_t = small_pool.tile([P, D], dt, tag="junk")
            part_t = small_pool.tile([P, 4 * J], dt, tag="part")
            tmp_t = small_pool.tile([P, BL], dt, tag="tmp")
            eps_t = small_pool.tile([P, 1], dt, tag="eps")
            nc.vector.memset(eps_t[:], EPS_PD)

            for b in range(BL):
                # [125, 8, 300] = the full contiguous 1.2 MB b-chunk in one
                # DMA: SP-sequencer issue cost is ~0.7 us per dma_start, so
                # few big DMAs beat many small ones.
                e_t = io_pool.tile([P, J, D], dt, tag="e")
                nc.sync.dma_start(e_t[:], emb_v[b])
                a_t = io_pool.tile([P, J, D], dt, tag="a")
                nc.sync.dma_start(a_t[:], att_v[b])
                d_t = dif_pool.tile([P, J, D], dt, tag="d")
                nc.vector.tensor_sub(d_t[:], e_t[:], a_t[:])
                for j in range(J):
                    # square(d + eps): torch pairwise_distance eps as ACT
                    # bias; accum_out gives the row-sum for free
                    nc.scalar.activation(
                        junk_t[:], d_t[:, j, :], Act.Square,
                        bias=eps_t[:], scale=1.0,
                        accum_out=rel_t[:, j * BL + b: j * BL + b + 1],
                    )

            for j in range(J):
                rel_j = rel_t[:, j * BL:(j + 1) * BL]
                m_j = mask_t[:, j * BL:(j + 1) * BL]
                m2_j = mask2_t[:, j * BL:(j + 1) * BL]
                nc.vector.tensor_reduce(
                    part_t[:, 0 * J + j: 0 * J + j + 1], rel_j,
                    axis=Ax.X, op=Alu.max)
                # masking via min/max with +-1e30 select masks is exact
                nc.vector.tensor_tensor(tmp_t[:], rel_j, m_j, op=Alu.min)
                nc.vector.tensor_reduce(
                    part_t[:, 1 * J + j: 1 * J + j + 1], tmp_t[:],
                    axis=Ax.X, op=Alu.max)
                nc.vector.tensor_tensor(tmp_t[:], rel_j, m_j, op=Alu.max)
                nc.vector.tensor_reduce(
                    part_t[:, 2 * J + j: 2 * J + j + 1], tmp_t[:],
                    axis=Ax.X, op=Alu.min)
                nc.vector.tensor_tensor(tmp_t[:], rel_j, m2_j, op=Alu.max)
                nc.vector.tensor_reduce(
                    part_t[:, 3 * J + j: 3 * J + j + 1], tmp_t[:],
                    axis=Ax.X, op=Alu.min)
            nc.sync.dma_start(out[:], part_t[:])
    nc.compile()
    return nc


def _get_nc():
    if "nc" not in _STATE:
        _STATE["nc"] = _build()
    return _STATE["nc"]


def _make_masks(labels_np):
    """Per-core select masks msk[p, j*BL+b] = +BIG if labels[b]==j*P+p else -BIG."""
    masks = []
    c_of_pj = np.arange(P)[:, None] + P * np.arange(J)[None, :]     # [P, J]
    for m in range(M):
        lb = labels_np[m * BL:(m + 1) * BL].astype(np.int64)        # [BL]
        match = c_of_pj[:, :, None] == lb[None, None, :]            # [P, J, BL]
        mask = np.where(match, np.float32(BIG), np.float32(-BIG))
        masks.append(np.ascontiguousarray(mask.reshape(P, J * BL),
                                          dtype=np.float32))
    return masks


def _run_device(attributes, embeddings, labels_np, trace=False):
    from concourse.bass_utils import run_bass_kernel_spmd
    nc = _get_nc()
    masks = _make_masks(labels_np)
    in_maps = []
    for m in range(M):
        sl = slice(m * BL * C, (m + 1) * BL * C)
        in_maps.append({
            "emb": embeddings[sl],
            "att": attributes[sl],
            "msk": masks[m],
        })
    return run_bass_kernel_spmd(nc, in_maps, list(range(M)), trace=trace)


def _combine(results):
    """All-reduce the per-core [P, 4J] partials and finish the loss on host."""
    cmax = np.full(C, -np.inf)
    pmax = np.full(C, -np.inf)
    umin = np.full(C, np.inf)
    mmin = np.full(C, np.inf)
    for m in range(M):
        o = results[m]["out"].astype(np.float64)       # [P, 4*J], col = k*J+j
        okjp = np.transpose(o.reshape(P, 4, J), (1, 2, 0)).reshape(4, C)
        cmax = np.maximum(cmax, okjp[0])
        pmax = np.maximum(pmax, okjp[1])
        umin = np.minimum(umin, okjp[2])
        mmin = np.minimum(mmin, okjp[3])
    # squared space -> distances (max/min commute with sqrt on [0, inf))
    mx = np.sqrt(np.maximum(cmax, 0.0))
    hp = np.sqrt(np.maximum(pmax, 0.0))       # -BIG (no positive) -> 0
    umin_r = np.sqrt(np.maximum(umin, 0.0))   # +BIG sentinel stays huge
    mmin_r = np.sqrt(np.maximum(mmin, 0.0))
    hn = np.minimum(umin_r, mx + mmin_r)
    triplet = np.maximum(hp - hn + MARGIN, 0.0)
    num_hard = np.sum(triplet > 1e-16)
    loss = np.sum(triplet) / (num_hard + 1e-16)
    return np.float32(loss)


def kernel(attributes, embeddings, labels):
    attributes = np.ascontiguousarray(np.asarray(attributes, dtype=np.float32))
    embeddings = np.ascontiguousarray(np.asarray(embeddings, dtype=np.float32))
    labels_np = np.asarray(labels)
    res = _run_device(attributes, embeddings, labels_np)
    return _combine(res.results)


# revision 14
# speedup vs baseline: 1.1364x; 1.1364x over previous
"""HardTripletLoss Trainium2 kernel.

Reference computation (B=256, C=1000, D=300):
  relations[b,c] = ||emb[b*C+c] - att[b*C+c] + 1e-6||_2          [B, C]
  hardest_positive[c] = max_b relations[b,c] * onehot(labels)[b,c]
  mx[c]              = max_b relations[b,c]
  hardest_negative[c] = min_b (relations[b,c] + mx[c]*onehot[b,c])
  loss = sum(relu(hp - hn + 1)) / (count(relu(...) > 1e-16) + 1e-16)

Sharding: data-parallel over B across 8 cores (32 b's per core, each a
contiguous 32000-row chunk of the (B*C, D) tensors). Each core computes
squared distances and 4 per-class partial reductions [C]:
  cmax = max_b rel_sq            pmax = max_b over positives of rel_sq
  umin = min_b over negatives    mmin = min_b over positives
(masking is exact: +/-1e30 select-style masks via min/max ALU ops, no
additive-cancellation error). Host all-reduces the [4, C] partials over
cores, takes sqrt (monotone, commutes with max/min), and finishes the
tiny [C]-sized tail: hn = min(umin, cmax_r + mmin), loss scalar.

On-chip layout per core: partition p holds 8 CONSECUTIVE rows of the
1000-row b-chunk (c = 8p + r, r in [0,8)), so each per-b DMA is a single
dense 1.2 MB 2D transfer with 9.6 KB contiguous per-partition lines --
this is what keeps the DMA engines at full packet efficiency (1.2 KB
strided lines ran at 15% MBU). Per b: 2 DMAs, one DVE subtract
[125, 2400], 8 ACT Square(bias=eps) ops with accum_out -> rel column.
"""

import numpy as np

B, C, D = 256, 1000, 300
M = 8            # cores
BL = B // M      # 32 local anchors per core
P = 125          # partitions; partition p holds classes c = 8p + r
R = C // P       # 8 consecutive rows per partition
BIG = 1.0e30
EPS_PD = 1e-6
MARGIN = 1.0

_STATE = {}


def _build():
    import concourse.tile as tile
    from concourse import bacc, mybir

    nc = bacc.Bacc("TRN2", target_bir_lowering=False, debug=False,
                   num_devices=M)
    dt = mybir.dt.float32
    emb = nc.dram_tensor("emb", [BL * C, D], dt, kind="ExternalInput").ap()
    att = nc.dram_tensor("att", [BL * C, D], dt, kind="ExternalInput").ap()
    msk = nc.dram_tensor("msk", [P, R * BL], dt, kind="ExternalInput").ap()
    out = nc.dram_tensor("out", [P, 4 * R], dt, kind="ExternalOutput").ap()

    emb_v = emb.rearrange("(b p r) d -> b p r d", b=BL, p=P, r=R)
    att_v = att.rearrange("(b p r) d -> b p r d", b=BL, p=P, r=R)

    Alu = mybir.AluOpType
    Act = mybir.ActivationFunctionType
    Ax = mybir.AxisListType

    with tile.TileContext(nc) as tc:
        with (
            tc.tile_pool(name="io", bufs=4) as io_pool,
            tc.tile_pool(name="dif", bufs=2) as dif_pool,
            tc.tile_pool(name="small", bufs=1) as small_pool,
        ):
            mask_t = small_pool.tile([P, R * BL], dt, tag="mask")
            nc.sync.dma_start(mask_t[:], msk[:])
            mask2_t = small_pool.tile([P, R * BL], dt, tag="mask2")
            nc.vector.tensor_scalar_mul(mask2_t[:], mask_t[:], -1.0)
            # rel_t column b*R + r holds rel_sq of (b, c=8p+r)
            rel_t = small_pool.tile([P, BL * R], dt, tag="rel")
            junk_t = small_pool.tile([P, D], dt, tag="junk")
            part_t = small_pool.tile([P, 4 * R], dt, tag="part")
            tmp_t = small_pool.tile([P, BL], dt, tag="tmp")
            eps_t = small_pool.tile([P, 1], dt, tag="eps")
            nc.vector.memset(eps_t[:], EPS_PD)

            for b in range(BL):
                # whole contiguous 1.2 MB b-chunk, 9.6 KB per partition line
                e_t = io_pool.tile([P, R, D], dt, tag="e")
                nc.sync.dma_start(e_t[:], emb_v[b])
                a_t = io_pool.tile([P, R, D], dt, tag="a")
                nc.sync.dma_start(a_t[:], att_v[b])
                d_t = dif_pool.tile([P, R, D], dt, tag="d")
                nc.vector.tensor_sub(d_t[:], e_t[:], a_t[:])
                for r in range(R):
                    # square(d + eps): torch pairwise_distance eps as ACT
                    # bias; accum_out gives the 300-wide row-sum for free
                    nc.scalar.activation(
                        junk_t[:], d_t[:, r, :], Act.Square,
                        bias=eps_t[:], scale=1.0,
                        accum_out=rel_t[:, b * R + r: b * R + r + 1],
                    )

            for r in range(R):
                # strided view: all b for this r (free stride R)
                rel_r = rel_t[:, r: BL * R: R]
                m_r = mask_t[:, r * BL:(r + 1) * BL]
                m2_r = mask2_t[:, r * BL:(r + 1) * BL]
                nc.vector.tensor_reduce(
                    part_t[:, 0 * R + r: 0 * R + r + 1], rel_r,
                    axis=Ax.X, op=Alu.max)
                # masking via min/max with +-1e30 select masks is exact
                nc.vector.tensor_tensor(tmp_t[:], rel_r, m_r, op=Alu.min)
                nc.vector.tensor_reduce(
                    part_t[:, 1 * R + r: 1 * R + r + 1], tmp_t[:],
                    axis=Ax.X, op=Alu.max)
                nc.vector.tensor_tensor(tmp_t[:], rel_r, m_r, op=Alu.max)
                nc.vector.tensor_reduce(
                    part_t[:, 2 * R + r: 2 * R + r + 1], tmp_t[:],
                    axis=Ax.X, op=Alu.min)
                nc.vector.tensor_tensor(tmp_t[:], rel_r, m2_r, op=Alu.max)
                nc.vector.tensor_reduce(
                    part_t[:, 3 * R + r: 3 * R + r + 1], tmp_t[:],
                    axis=Ax.X, op=Alu.min)
            nc.sync.dma_start(out[:], part_t[:])
    nc.compile()
    return nc


def _get_nc():
    if "nc" not in _STATE:
        _STATE["nc"] = _build()
    return _STATE["nc"]


def _make_masks(labels_np):
    """Per-core select masks msk[p, r*BL+b] = +BIG if labels[b]==8p+r else -BIG."""
    masks = []
    c_of_pr = R * np.arange(P)[:, None] + np.arange(R)[None, :]     # [P, R]
    for m in range(M):
        lb = labels_np[m * BL:(m + 1) * BL].astype(np.int64)        # [BL]
        match = c_of_pr[:, :, None] == lb[None, None, :]            # [P, R, BL]
        mask = np.where(match, np.float32(BIG), np.float32(-BIG))
        masks.append(np.ascontiguousarray(mask.reshape(P, R * BL),
                                          dtype=np.float32))
    return masks


def _partials_from_out(o):
    """Device out [P, 4R] (col k*R+r, class c = R*p + r) -> [4, C] float64."""
    return np.transpose(o.astype(np.float64).reshape(P, 4, R),
                        (1, 0, 2)).reshape(4, C)


def _run_device(attributes, embeddings, labels_np, trace=False):
    from concourse.bass_utils import run_bass_kernel_spmd
    nc = _get_nc()
    masks = _make_masks(labels_np)
    in_maps = []
    for m in range(M):
        sl = slice(m * BL * C, (m + 1) * BL * C)
        in_maps.append({
            "emb": embeddings[sl],
            "att": attributes[sl],
            "msk": masks[m],
        })
    return run_bass_kernel_spmd(nc, in_maps, list(range(M)), trace=trace)


def _combine(results):
    """All-reduce the per-core [P, 4R] partials and finish the loss on host."""
    cmax = np.full(C, -np.inf)
    pmax = np.full(C, -np.inf)
    umin = np.full(C, np.inf)
    mmin = np.full(C, np.inf)
    for m in range(M):
        pk = _partials_from_out(results[m]["out"])
        cmax = np.maximum(cmax, pk[0])
        pmax = np.maximum(pmax, pk[1])
        umin = np.minimum(umin, pk[2])
        mmin = np.minimum(mmin, pk[3])
    # squared space -> distances (max/min commute with sqrt on [0, inf))
    mx = np.sqrt(np.maximum(cmax, 0.0))
    hp = np.sqrt(np.maximum(pmax, 0.0))       # -BIG (no positive) -> 0
    umin_r = np.sqrt(np.maximum(umin, 0.0))   # +BIG sentinel stays huge
    mmin_r = np.sqrt(np.maximum(mmin, 0.0))
    hn = np.minimum(umin_r, mx + mmin_r)
    triplet = np.maximum(hp - hn + MARGIN, 0.0)
    num_hard = np.sum(triplet > 1e-16)
    loss = np.sum(triplet) / (num_hard + 1e-16)
    return np.float32(loss)


def kernel(attributes, embeddings, labels):
    attributes = np.ascontiguousarray(np.asarray(attributes, dtype=np.float32))
    embeddings = np.ascontiguousarray(np.asarray(embeddings, dtype=np.float32))
    labels_np = np.asarray(labels)
    res = _run_device(attributes, embeddings, labels_np)
    return _combine(res.results)


# revision 15
# speedup vs baseline: 1.1481x; 1.0103x over previous
"""HardTripletLoss Trainium2 kernel.

Reference computation (B=256, C=1000, D=300):
  relations[b,c] = ||emb[b*C+c] - att[b*C+c] + 1e-6||_2          [B, C]
  hardest_positive[c] = max_b relations[b,c] * onehot(labels)[b,c]
  mx[c]              = max_b relations[b,c]
  hardest_negative[c] = min_b (relations[b,c] + mx[c]*onehot[b,c])
  loss = sum(relu(hp - hn + 1)) / (count(relu(...) > 1e-16) + 1e-16)

Sharding: data-parallel over B across 8 cores (32 b's per core, each a
contiguous 32000-row chunk of the (B*C, D) tensors). Each core computes
squared distances and 4 per-class partial reductions [C]:
  cmax = max_b rel_sq            pmax = max_b over positives of rel_sq
  umin = min_b over negatives    mmin = min_b over positives
(masking is exact: +/-1e30 select-style masks via min/max ALU ops, no
additive-cancellation error). Host all-reduces the [4, C] partials over
cores, takes sqrt (monotone, commutes with max/min), and finishes the
tiny [C]-sized tail: hn = min(umin, cmax_r + mmin), loss scalar.

On-chip layout per core: partition p holds 8 CONSECUTIVE rows of the
1000-row b-chunk (c = 8p + r, r in [0,8)), so each per-b DMA is a single
dense 1.2 MB 2D transfer with 9.6 KB contiguous per-partition lines --
this is what keeps the DMA engines at full packet efficiency (1.2 KB
strided lines ran at 15% MBU). Per b: 2 DMAs, one DVE subtract
[125, 2400], 8 ACT Square(bias=eps) ops with accum_out -> rel column.
"""

import numpy as np

B, C, D = 256, 1000, 300
M = 8            # cores
BL = B // M      # 32 local anchors per core
P = 125          # partitions; partition p holds classes c = 8p + r
R = C // P       # 8 consecutive rows per partition
BIG = 1.0e30
EPS_PD = 1e-6
MARGIN = 1.0

_STATE = {}


def _build():
    import concourse.tile as tile
    from concourse import bacc, mybir

    nc = bacc.Bacc("TRN2", target_bir_lowering=False, debug=False,
                   num_devices=M)
    dt = mybir.dt.float32
    emb = nc.dram_tensor("emb", [BL * C, D], dt, kind="ExternalInput").ap()
    att = nc.dram_tensor("att", [BL * C, D], dt, kind="ExternalInput").ap()
    msk = nc.dram_tensor("msk", [P, R * BL], dt, kind="ExternalInput").ap()
    out = nc.dram_tensor("out", [P, 4 * R], dt, kind="ExternalOutput").ap()

    emb_v = emb.rearrange("(b p r) d -> b p r d", b=BL, p=P, r=R)
    att_v = att.rearrange("(b p r) d -> b p r d", b=BL, p=P, r=R)

    Alu = mybir.AluOpType
    Act = mybir.ActivationFunctionType
    Ax = mybir.AxisListType

    with tile.TileContext(nc) as tc:
        with (
            tc.tile_pool(name="io", bufs=4) as io_pool,
            tc.tile_pool(name="dif", bufs=2) as dif_pool,
            tc.tile_pool(name="small", bufs=1) as small_pool,
        ):
            mask_t = small_pool.tile([P, R * BL], dt, tag="mask")
            nc.sync.dma_start(mask_t[:], msk[:])
            mask2_t = small_pool.tile([P, R * BL], dt, tag="mask2")
            nc.vector.tensor_scalar_mul(mask2_t[:], mask_t[:], -1.0)
            # rel_t column b*R + r holds rel_sq of (b, c=8p+r)
            rel_t = small_pool.tile([P, BL * R], dt, tag="rel")
            junk_t = small_pool.tile([P, D], dt, tag="junk")
            part_t = small_pool.tile([P, 4 * R], dt, tag="part")
            tmp_t = small_pool.tile([P, BL], dt, tag="tmp")
            eps_t = small_pool.tile([P, 1], dt, tag="eps")
            nc.vector.memset(eps_t[:], EPS_PD)

            for b in range(BL):
                # whole contiguous 1.2 MB b-chunk, 9.6 KB per partition line
                # split loads across BOTH HWDGE queues (qSyncDynamicHW +
                # qScalarDynamicHW): one queue alone caps at ~134 GB/s
                e_t = io_pool.tile([P, R, D], dt, tag="e")
                nc.sync.dma_start(e_t[:], emb_v[b])
                a_t = io_pool.tile([P, R, D], dt, tag="a")
                nc.scalar.dma_start(a_t[:], att_v[b])
                d_t = dif_pool.tile([P, R, D], dt, tag="d")
                nc.vector.tensor_sub(d_t[:], e_t[:], a_t[:])
                for r in range(R):
                    # square(d + eps): torch pairwise_distance eps as ACT
                    # bias; accum_out gives the 300-wide row-sum for free
                    nc.scalar.activation(
                        junk_t[:], d_t[:, r, :], Act.Square,
                        bias=eps_t[:], scale=1.0,
                        accum_out=rel_t[:, b * R + r: b * R + r + 1],
                    )

            for r in range(R):
                # strided view: all b for this r (free stride R)
                rel_r = rel_t[:, r: BL * R: R]
                m_r = mask_t[:, r * BL:(r + 1) * BL]
                m2_r = mask2_t[:, r * BL:(r + 1) * BL]
                nc.vector.tensor_reduce(
                    part_t[:, 0 * R + r: 0 * R + r + 1], rel_r,
                    axis=Ax.X, op=Alu.max)
                # masking via min/max with +-1e30 select masks is exact
                nc.vector.tensor_tensor(tmp_t[:], rel_r, m_r, op=Alu.min)
                nc.vector.tensor_reduce(
                    part_t[:, 1 * R + r: 1 * R + r + 1], tmp_t[:],
                    axis=Ax.X, op=Alu.max)
                nc.vector.tensor_tensor(tmp_t[:], rel_r, m_r, op=Alu.max)
                nc.vector.tensor_reduce(
                    part_t[:, 2 * R + r: 2 * R + r + 1], tmp_t[:],
                    axis=Ax.X, op=Alu.min)
                nc.vector.tensor_tensor(tmp_t[:], rel_r, m2_r, op=Alu.max)
                nc.vector.tensor_reduce(
                    part_t[:, 3 * R + r: 3 * R + r + 1], tmp_t[:],
                    axis=Ax.X, op=Alu.min)
            nc.sync.dma_start(out[:], part_t[:])
    nc.compile()
    return nc


def _get_nc():
    if "nc" not in _STATE:
        _STATE["nc"] = _build()
    return _STATE["nc"]


def _make_masks(labels_np):
    """Per-core select masks msk[p, r*BL+b] = +BIG if labels[b]==8p+r else -BIG."""
    masks = []
    c_of_pr = R * np.arange(P)[:, None] + np.arange(R)[None, :]     # [P, R]
    for m in range(M):
        lb = labels_np[m * BL:(m + 1) * BL].astype(np.int64)        # [BL]
        match = c_of_pr[:, :, None] == lb[None, None, :]            # [P, R, BL]
        mask = np.where(match, np.float32(BIG), np.float32(-BIG))
        masks.append(np.ascontiguousarray(mask.reshape(P, R * BL),
                                          dtype=np.float32))
    return masks


def _partials_from_out(o):
    """Device out [P, 4R] (col k*R+r, class c = R*p + r) -> [4, C] float64."""
    return np.transpose(o.astype(np.float64).reshape(P, 4, R),
                        (1, 0, 2)).reshape(4, C)


def _run_device(attributes, embeddings, labels_np, trace=False):
    from concourse.bass_utils import run_bass_kernel_spmd
    nc = _get_nc()
    masks = _make_masks(labels_np)
    in_maps = []
    for m in range(M):
        sl = slice(m * BL * C, (m + 1) * BL * C)
        in_maps.append({
            "emb": embeddings[sl],
            "att": attributes[sl],
            "msk": masks[m],
        })
    return run_bass_kernel_spmd(nc, in_maps, list(range(M)), trace=trace)


def _combine(results):
    """All-reduce the per-core [P, 4R] partials and finish the loss on host."""
    cmax = np.full(C, -np.inf)
    pmax = np.full(C, -np.inf)
    umin = np.full(C, np.inf)
    mmin = np.full(C, np.inf)
    for m in range(M):
        pk = _partials_from_out(results[m]["out"])
        cmax = np.maximum(cmax, pk[0])
        pmax = np.maximum(pmax, pk[1])
        umin = np.minimum(umin, pk[2])
        mmin = np.minimum(mmin, pk[3])
    # squared space -> distances (max/min commute with sqrt on [0, inf))
    mx = np.sqrt(np.maximum(cmax, 0.0))
    hp = np.sqrt(np.maximum(pmax, 0.0))       # -BIG (no positive) -> 0
    umin_r = np.sqrt(np.maximum(umin, 0.0))   # +BIG sentinel stays huge
    mmin_r = np.sqrt(np.maximum(mmin, 0.0))
    hn = np.minimum(umin_r, mx + mmin_r)
    triplet = np.maximum(hp - hn + MARGIN, 0.0)
    num_hard = np.sum(triplet > 1e-16)
    loss = np.sum(triplet) / (num_hard + 1e-16)
    return np.float32(loss)


def kernel(attributes, embeddings, labels):
    attributes = np.ascontiguousarray(np.asarray(attributes, dtype=np.float32))
    embeddings = np.ascontiguousarray(np.asarray(embeddings, dtype=np.float32))
    labels_np = np.asarray(labels)
    res = _run_device(attributes, embeddings, labels_np)
    return _combine(res.results)
